# revision 1
# baseline (speedup 1.0000x reference)
"""LIF spiking-neuron recurrence on Trainium2 (8 NeuronCores).

Reference semantics (TAU=1, THRESH=1, f32):
    mem = 0
    for t in range(T):
        mem = mem + x[t]
        spike[t] = (mem >= 1.0) ? 1.0 : 0.0
        mem = mem * (1 - spike[t])        # hard reset

Sharding: data-parallel over the batch axis (B=128 -> 16 rows/core).
Per-core layout: the [T, 16, 16384] shard is viewed as [T, 128, 2048]
(partition-major within a timestep slab) and pre-transposed on the host
to [128, T, 2048] so each partition's DMA runs are contiguous.

Engine mapping per timestep (tile [128, 2048] f32):
    DVE : tmp = mem + x_t                     (tensor_tensor add, 1x)
    POOL: spike = (tmp >= 1)                  (tensor_scalar is_ge)
    DVE : mem = (tmp < 1) * tmp               (scalar_tensor_tensor)
DVE never enters a 2-port perf mode (only 2-src ops), so it does not
contend with GpSimd for the shared SBUF port pair. DMAs are HWDGE
(loads on SP ring, stores on ACT ring), CHUNK timesteps per transfer.
"""

import numpy as np

try:
    import concourse  # noqa: F401
except ImportError:  # pragma: no cover
    import sys

    for _p in ("/opt/trn_rl_repo", "/root/.axon_site/_ro/trn_rl_repo"):
        if _p not in sys.path:
            sys.path.insert(0, _p)

from concourse import bacc, mybir
from concourse.bass_utils import run_bass_kernel_spmd
from concourse.mybir import AluOpType
from concourse.tile import TileContext

T, B, D = 64, 128, 16384
NCORES = 8
BL = B // NCORES  # 16 batch rows per core
P = 128  # SBUF partitions
F = (BL * D) // P  # 2048 free elements per timestep slab
CHUNK = 4  # timesteps per DMA transfer


def build_nc(t_steps=T, f_free=F, chunk=CHUNK, x_bufs=2, s_bufs=2, t_bufs=3):
    """Build + compile the per-core Bass program (identical on all cores)."""
    assert t_steps % chunk == 0
    f32 = mybir.dt.float32
    nc = bacc.Bacc(
        "TRN2", target_bir_lowering=False, debug=False, num_devices=NCORES
    )
    x_ext = nc.dram_tensor("x", [P, t_steps, f_free], f32, kind="ExternalInput")
    out_ext = nc.dram_tensor(
        "out", [P, t_steps, f_free], f32, kind="ExternalOutput"
    )
    with TileContext(nc) as tc:
        with (
            tc.tile_pool(name="xp", bufs=x_bufs) as xp,
            tc.tile_pool(name="sp", bufs=s_bufs) as sp,
            tc.tile_pool(name="tp", bufs=t_bufs) as tp,
            tc.tile_pool(name="mp", bufs=1) as mp,
        ):
            mem = mp.tile([P, f_free], f32)
            nc.vector.memset(mem[:], 0.0)
            for g in range(t_steps // chunk):
                xt = xp.tile([P, chunk * f_free], f32, name="xt")
                nc.sync.dma_start(
                    xt[:],
                    x_ext[:, g * chunk : (g + 1) * chunk, :].rearrange(
                        "p t f -> p (t f)"
                    ),
                )
                spk = sp.tile([P, chunk * f_free], f32, name="spk")
                for j in range(chunk):
                    xs = xt[:, j * f_free : (j + 1) * f_free]
                    ss = spk[:, j * f_free : (j + 1) * f_free]
                    tmp = tp.tile([P, f_free], f32, name="tmp")
                    nc.vector.tensor_tensor(tmp[:], mem[:], xs, AluOpType.add)
                    nc.gpsimd.tensor_scalar(ss, tmp[:], 1.0, None, AluOpType.is_ge)
                    nc.vector.scalar_tensor_tensor(
                        mem[:], tmp[:], 1.0, tmp[:], AluOpType.is_lt, AluOpType.mult
                    )
                nc.scalar.dma_start(
                    out_ext[:, g * chunk : (g + 1) * chunk, :].rearrange(
                        "p t f -> p (t f)"
                    ),
                    spk[:],
                )
    nc.compile()
    return nc


_cached_nc = None


def _get_nc():
    global _cached_nc
    if _cached_nc is None:
        _cached_nc = build_nc()
    return _cached_nc


def _shard(x):
    """Full [T, B, D] -> list of per-core [P, T, F] contiguous arrays."""
    in_maps = []
    for c in range(NCORES):
        xc = x[:, c * BL : (c + 1) * BL, :].reshape(T, P, F).transpose(1, 0, 2)
        in_maps.append({"x": np.ascontiguousarray(xc)})
    return in_maps


def _gather(results):
    """Per-core [P, T, F] outputs -> full [T, B, D]."""
    outs = [
        np.asarray(results[c]["out"]).transpose(1, 0, 2).reshape(T, BL, D)
        for c in range(NCORES)
    ]
    return np.concatenate(outs, axis=1)


def run(x, trace=False, **kw):
    """Run on the 8 NeuronCores; returns (output, BassKernelResults)."""
    x = np.ascontiguousarray(np.asarray(x, dtype=np.float32))
    assert x.shape == (T, B, D), x.shape
    nc = _get_nc()
    res = run_bass_kernel_spmd(
        nc, _shard(x), core_ids=list(range(NCORES)), trace=trace, **kw
    )
    return _gather(res.results), res


def kernel(x: np.ndarray) -> np.ndarray:
    out, _ = run(x)
    return out


# revision 3
# speedup vs baseline: 5.4262x; 5.4262x over previous
"""LIF spiking-neuron recurrence on Trainium2 (8 NeuronCores).

Reference semantics (TAU=1, THRESH=1, f32):
    mem = 0
    for t in range(T):
        mem = mem + x[t]
        spike[t] = (mem >= 1.0) ? 1.0 : 0.0
        mem = mem * (1 - spike[t])        # hard reset

Sharding: data-parallel over the batch axis (B=128 -> 16 rows/core).
Per-core layout: the [T, 16, 16384] shard is viewed as [T, 128, 2048]
(partition-major within a timestep slab) and pre-transposed on the host
to [128, T, 2048] so each partition's DMA runs are contiguous.

Engine mapping per timestep (tile [128, 2048] f32), all on DVE
(GpSimd f32 elementwise measured ~15-30x below DVE rate, and its
shared-port lock stalls DVE tensor_tensor - keep GpSimd idle):
    DVE : tmp = mem + x_t            (tensor_tensor add, 1x, ~2.29us)
    DVE : spike = (tmp >= 1)         (tensor_scalar is_ge, 2x, ~1.15us)
    DVE : mem = (tmp < 1) * tmp      (scalar_tensor_tensor, 1x, ~2.29us)
DMAs are HWDGE (loads on SP ring, stores on ACT ring), CHUNK timesteps
per transfer. Step 0 uses x_0 directly (mem starts at 0); the final
step's reset is dead code and skipped.
"""

import numpy as np

try:
    import concourse  # noqa: F401
except ImportError:  # pragma: no cover
    import sys

    for _p in ("/opt/trn_rl_repo", "/root/.axon_site/_ro/trn_rl_repo"):
        if _p not in sys.path:
            sys.path.insert(0, _p)

from concourse import bacc, mybir
from concourse.bass_utils import run_bass_kernel_spmd
from concourse.mybir import AluOpType
from concourse.tile import TileContext

T, B, D = 64, 128, 16384
NCORES = 8
BL = B // NCORES  # 16 batch rows per core
P = 128  # SBUF partitions
F = (BL * D) // P  # 2048 free elements per timestep slab
CHUNK = 4  # timesteps per DMA transfer


def build_nc(t_steps=T, f_free=F, chunk=CHUNK, x_bufs=2, s_bufs=2, t_bufs=3):
    """Build + compile the per-core Bass program (identical on all cores)."""
    assert t_steps % chunk == 0
    f32 = mybir.dt.float32
    nc = bacc.Bacc(
        "TRN2", target_bir_lowering=False, debug=False, num_devices=NCORES
    )
    x_ext = nc.dram_tensor("x", [P, t_steps, f_free], f32, kind="ExternalInput")
    out_ext = nc.dram_tensor(
        "out", [P, t_steps, f_free], f32, kind="ExternalOutput"
    )
    with TileContext(nc) as tc:
        with (
            tc.tile_pool(name="xp", bufs=x_bufs) as xp,
            tc.tile_pool(name="sp", bufs=s_bufs) as sp,
            tc.tile_pool(name="tp", bufs=t_bufs) as tp,
            tc.tile_pool(name="mp", bufs=1) as mp,
        ):
            mem = mp.tile([P, f_free], f32)
            for g in range(t_steps // chunk):
                xt = xp.tile([P, chunk * f_free], f32, name="xt")
                nc.sync.dma_start(
                    xt[:],
                    x_ext[:, g * chunk : (g + 1) * chunk, :].rearrange(
                        "p t f -> p (t f)"
                    ),
                )
                spk = sp.tile([P, chunk * f_free], f32, name="spk")
                for j in range(chunk):
                    t = g * chunk + j
                    xs = xt[:, j * f_free : (j + 1) * f_free]
                    ss = spk[:, j * f_free : (j + 1) * f_free]
                    if t == 0:
                        pre = xs  # mem==0: pre-reset membrane is just x_0
                    else:
                        tmp = tp.tile([P, f_free], f32, name="tmp")
                        nc.vector.tensor_tensor(
                            tmp[:], mem[:], xs, AluOpType.add
                        )
                        pre = tmp[:]
                    nc.vector.tensor_scalar(ss, pre, 1.0, None, AluOpType.is_ge)
                    if t < t_steps - 1:  # last reset is dead code
                        nc.vector.scalar_tensor_tensor(
                            mem[:], pre, 1.0, pre, AluOpType.is_lt, AluOpType.mult
                        )
                nc.scalar.dma_start(
                    out_ext[:, g * chunk : (g + 1) * chunk, :].rearrange(
                        "p t f -> p (t f)"
                    ),
                    spk[:],
                )
    nc.compile()
    return nc


_cached_nc = None


def _get_nc():
    global _cached_nc
    if _cached_nc is None:
        _cached_nc = build_nc()
    return _cached_nc


def _shard(x):
    """Full [T, B, D] -> list of per-core [P, T, F] contiguous arrays."""
    in_maps = []
    for c in range(NCORES):
        xc = x[:, c * BL : (c + 1) * BL, :].reshape(T, P, F).transpose(1, 0, 2)
        in_maps.append({"x": np.ascontiguousarray(xc)})
    return in_maps


def _gather(results):
    """Per-core [P, T, F] outputs -> full [T, B, D]."""
    outs = [
        np.asarray(results[c]["out"]).transpose(1, 0, 2).reshape(T, BL, D)
        for c in range(NCORES)
    ]
    return np.concatenate(outs, axis=1)


def run(x, trace=False, **kw):
    """Run on the 8 NeuronCores; returns (output, BassKernelResults)."""
    x = np.ascontiguousarray(np.asarray(x, dtype=np.float32))
    assert x.shape == (T, B, D), x.shape
    nc = _get_nc()
    res = run_bass_kernel_spmd(
        nc, _shard(x), core_ids=list(range(NCORES)), trace=trace, **kw
    )
    return _gather(res.results), res


def kernel(x: np.ndarray) -> np.ndarray:
    out, _ = run(x)
    return out


# revision 7
# speedup vs baseline: 6.0941x; 1.1231x over previous
"""LIF spiking-neuron recurrence on Trainium2 (8 NeuronCores).

Reference semantics (TAU=1, THRESH=1, f32):
    mem = 0
    for t in range(T):
        mem = mem + x[t]
        spike[t] = (mem >= 1.0) ? 1.0 : 0.0
        mem = mem * (1 - spike[t])        # hard reset

Sharding: data-parallel over the batch axis (B=128 -> 16 rows/core).
Per-core layout: the [T, 16, 16384] shard is viewed as [T, 128, 2048]
(partition-major within a timestep slab) and pre-transposed on the host
to [128, T, 2048] so each partition's DMA runs are contiguous.

Engine mapping per timestep (tile [128, 2048] f32):
    DVE : tmp = mem + x_t            (tensor_tensor add, 1x, ~2.29us)
    ACT : d = Sqrt(tmp + (-1))       (NaN iff tmp < 1; affine is exact)
    ACT : spike = Is_finite(d)       (exact 1.0/0.0, written as bf16)
    DVE : mem = (tmp < 1) * tmp      (scalar_tensor_tensor, 1x, ~2.29us)
The ACT spike route was probed exact on HW for all threshold edge
cases (ties, +-1ulp); GpSimd is kept idle (f32 elementwise there runs
~15-30x below DVE and its shared-port lock stalls DVE). Spikes are
stored as bf16 (0/1 exact) halving store traffic; the host upcasts.
DMAs are HWDGE (loads on SP ring, stores on ACT ring), CHUNK timesteps
per transfer. Step 0 uses x_0 directly (mem starts at 0); the final
step's reset is dead code and skipped.
"""

import numpy as np

try:
    import concourse  # noqa: F401
except ImportError:  # pragma: no cover
    import sys

    for _p in ("/opt/trn_rl_repo", "/root/.axon_site/_ro/trn_rl_repo"):
        if _p not in sys.path:
            sys.path.insert(0, _p)

from concourse import bacc, mybir
from concourse.bass_utils import run_bass_kernel_spmd
from concourse.mybir import ActivationFunctionType as AF
from concourse.mybir import AluOpType
from concourse.tile import TileContext

T, B, D = 64, 128, 16384
NCORES = 8
BL = B // NCORES  # 16 batch rows per core
P = 128  # SBUF partitions
F = (BL * D) // P  # 2048 free elements per timestep slab
CHUNK = 4  # timesteps per DMA transfer


def build_nc(
    t_steps=T, f_free=F, chunk=CHUNK, x_bufs=3, s_bufs=2, t_bufs=3, d_bufs=2
):
    """Build + compile the per-core Bass program (identical on all cores)."""
    assert t_steps % chunk == 0
    f32 = mybir.dt.float32
    bf16 = mybir.dt.bfloat16
    nc = bacc.Bacc(
        "TRN2", target_bir_lowering=False, debug=False, num_devices=NCORES
    )
    x_ext = nc.dram_tensor("x", [P, t_steps, f_free], f32, kind="ExternalInput")
    out_ext = nc.dram_tensor(
        "out", [P, t_steps, f_free], bf16, kind="ExternalOutput"
    )
    with TileContext(nc) as tc:
        with (
            tc.tile_pool(name="xp", bufs=x_bufs) as xp,
            tc.tile_pool(name="sp", bufs=s_bufs) as sp,
            tc.tile_pool(name="tp", bufs=t_bufs) as tp,
            tc.tile_pool(name="dp", bufs=d_bufs) as dp,
            tc.tile_pool(name="mp", bufs=1) as mp,
        ):
            mem = mp.tile([P, f_free], f32)
            bm1 = mp.tile([P, 1], f32, name="bm1")
            nc.vector.memset(bm1[:], -1.0)
            for g in range(t_steps // chunk):
                xt = xp.tile([P, chunk * f_free], f32, name="xt")
                nc.sync.dma_start(
                    xt[:],
                    x_ext[:, g * chunk : (g + 1) * chunk, :].rearrange(
                        "p t f -> p (t f)"
                    ),
                )
                spk = sp.tile([P, chunk * f_free], bf16, name="spk")
                for j in range(chunk):
                    t = g * chunk + j
                    xs = xt[:, j * f_free : (j + 1) * f_free]
                    ss = spk[:, j * f_free : (j + 1) * f_free]
                    if t == 0:
                        pre = xs  # mem==0: pre-reset membrane is just x_0
                    else:
                        tmp = tp.tile([P, f_free], f32, name="tmp")
                        nc.vector.tensor_tensor(
                            tmp[:], mem[:], xs, AluOpType.add
                        )
                        pre = tmp[:]
                    # spike = Is_finite(Sqrt(pre - 1)): NaN iff pre < 1
                    d = dp.tile([P, f_free], f32, name="d")
                    nc.scalar.activation(
                        d[:], pre, AF.Sqrt, bias=bm1[:], scale=1.0
                    )
                    nc.scalar.activation(
                        ss, d[:], AF.Is_finite, bias=0.0, scale=1.0
                    )
                    if t < t_steps - 1:  # last reset is dead code
                        nc.vector.scalar_tensor_tensor(
                            mem[:], pre, 1.0, pre, AluOpType.is_lt, AluOpType.mult
                        )
                nc.scalar.dma_start(
                    out_ext[:, g * chunk : (g + 1) * chunk, :].rearrange(
                        "p t f -> p (t f)"
                    ),
                    spk[:],
                )
    nc.compile()
    return nc


_cached_nc = None


def _get_nc():
    global _cached_nc
    if _cached_nc is None:
        _cached_nc = build_nc()
    return _cached_nc


def _shard(x):
    """Full [T, B, D] -> list of per-core [P, T, F] contiguous arrays."""
    in_maps = []
    for c in range(NCORES):
        xc = x[:, c * BL : (c + 1) * BL, :].reshape(T, P, F).transpose(1, 0, 2)
        in_maps.append({"x": np.ascontiguousarray(xc)})
    return in_maps


def _gather(results):
    """Per-core [P, T, F] bf16 outputs -> full [T, B, D] f32 (exact)."""
    outs = [
        np.asarray(results[c]["out"])
        .astype(np.float32)
        .transpose(1, 0, 2)
        .reshape(T, BL, D)
        for c in range(NCORES)
    ]
    return np.concatenate(outs, axis=1)


def run(x, trace=False, **kw):
    """Run on the 8 NeuronCores; returns (output, BassKernelResults)."""
    x = np.ascontiguousarray(np.asarray(x, dtype=np.float32))
    assert x.shape == (T, B, D), x.shape
    nc = _get_nc()
    res = run_bass_kernel_spmd(
        nc, _shard(x), core_ids=list(range(NCORES)), trace=trace, **kw
    )
    return _gather(res.results), res


def kernel(x: np.ndarray) -> np.ndarray:
    out, _ = run(x)
    return out


# revision 10
# speedup vs baseline: 6.9035x; 1.1328x over previous
"""LIF spiking-neuron recurrence on Trainium2 (8 NeuronCores).

Reference semantics (TAU=1, THRESH=1, f32):
    mem = 0
    for t in range(T):
        mem = mem + x[t]
        spike[t] = (mem >= 1.0) ? 1.0 : 0.0
        mem = mem * (1 - spike[t])        # hard reset

Sharding: data-parallel over the batch axis (B=128 -> 16 rows/core).
Per-core layout: the [T, 16, 16384] shard is viewed as [T, 128, 2048]
(partition-major within a timestep slab) and pre-transposed on the host
to [128, T, 2048] so each partition's DMA runs are contiguous.

Engine mapping per timestep (tile [128, 2048] f32):
    DVE : tmp = mem + x_t            (tensor_tensor add, 1x, ~2.29us)
    ACT : d = Sqrt(tmp + (-1))       (NaN iff tmp < 1; affine is exact)
    ACT : spike = Is_finite(d)       (exact 1.0/0.0, written as bf16)
    DVE : mem = (tmp < 1) * tmp      (scalar_tensor_tensor, 1x, ~2.29us)
The ACT spike route was probed exact on HW for all threshold edge
cases (ties, +-1ulp); GpSimd is kept idle (f32 elementwise there runs
~15-30x below DVE and its shared-port lock stalls DVE). Spikes are
stored as uint8 (0/1 exact, probed) cutting store traffic 4x; the
host upcasts. DMAs are HWDGE (loads on SP ring, stores on ACT ring),
CHUNK timesteps per transfer; the first group loads per-step (1MB) so
compute starts early, and the last group stores per-step to shorten
the tail. Step 0 uses x_0 directly (mem starts at 0); the final
step's reset is dead code and skipped.
"""

import numpy as np

try:
    import concourse  # noqa: F401
except ImportError:  # pragma: no cover
    import sys

    for _p in ("/opt/trn_rl_repo", "/root/.axon_site/_ro/trn_rl_repo"):
        if _p not in sys.path:
            sys.path.insert(0, _p)

from concourse import bacc, mybir
from concourse.bass_utils import run_bass_kernel_spmd
from concourse.mybir import ActivationFunctionType as AF
from concourse.mybir import AluOpType
from concourse.tile import TileContext

T, B, D = 64, 128, 16384
NCORES = 8
BL = B // NCORES  # 16 batch rows per core
P = 128  # SBUF partitions
F = (BL * D) // P  # 2048 free elements per timestep slab
CHUNK = 4  # timesteps per DMA transfer


def build_nc(
    t_steps=T, f_free=F, chunk=CHUNK, x_bufs=4, s_bufs=2, t_bufs=3, d_bufs=2
):
    """Build + compile the per-core Bass program (identical on all cores)."""
    assert t_steps % chunk == 0
    f32 = mybir.dt.float32
    u8 = mybir.dt.uint8
    nc = bacc.Bacc(
        "TRN2", target_bir_lowering=False, debug=False, num_devices=NCORES
    )
    x_ext = nc.dram_tensor("x", [P, t_steps, f_free], f32, kind="ExternalInput")
    out_ext = nc.dram_tensor(
        "out", [P, t_steps, f_free], u8, kind="ExternalOutput"
    )
    n_groups = t_steps // chunk
    with TileContext(nc) as tc:
        with (
            tc.tile_pool(name="xp", bufs=x_bufs) as xp,
            tc.tile_pool(name="sp", bufs=s_bufs) as sp,
            tc.tile_pool(name="tp", bufs=t_bufs) as tp,
            tc.tile_pool(name="dp", bufs=d_bufs) as dp,
            tc.tile_pool(name="mp", bufs=1) as mp,
        ):
            mem = mp.tile([P, f_free], f32)
            bm1 = mp.tile([P, 1], f32, name="bm1")
            nc.vector.memset(bm1[:], -1.0)
            for g in range(n_groups):
                xt = xp.tile([P, chunk * f_free], f32, name="xt")
                xv = x_ext[:, g * chunk : (g + 1) * chunk, :]
                if g == 0:
                    # per-step loads so step 0 can start after ~1MB
                    for j in range(chunk):
                        nc.sync.dma_start(
                            xt[:, j * f_free : (j + 1) * f_free], xv[:, j, :]
                        )
                else:
                    nc.sync.dma_start(xt[:], xv.rearrange("p t f -> p (t f)"))
                spk = sp.tile([P, chunk * f_free], u8, name="spk")
                for j in range(chunk):
                    t = g * chunk + j
                    xs = xt[:, j * f_free : (j + 1) * f_free]
                    ss = spk[:, j * f_free : (j + 1) * f_free]
                    if t == 0:
                        pre = xs  # mem==0: pre-reset membrane is just x_0
                    else:
                        tmp = tp.tile([P, f_free], f32, name="tmp")
                        nc.vector.tensor_tensor(
                            tmp[:], mem[:], xs, AluOpType.add
                        )
                        pre = tmp[:]
                    # spike = Is_finite(Sqrt(pre - 1)): NaN iff pre < 1
                    d = dp.tile([P, f_free], f32, name="d")
                    nc.scalar.activation(
                        d[:], pre, AF.Sqrt, bias=bm1[:], scale=1.0
                    )
                    nc.scalar.activation(
                        ss, d[:], AF.Is_finite, bias=0.0, scale=1.0
                    )
                    if t < t_steps - 1:  # last reset is dead code
                        nc.vector.scalar_tensor_tensor(
                            mem[:], pre, 1.0, pre, AluOpType.is_lt, AluOpType.mult
                        )
                    if g == n_groups - 1:
                        # per-step stores so the tail drains quickly
                        nc.scalar.dma_start(
                            out_ext[:, g * chunk + j, :], ss
                        )
                if g < n_groups - 1:
                    nc.scalar.dma_start(
                        out_ext[:, g * chunk : (g + 1) * chunk, :].rearrange(
                            "p t f -> p (t f)"
                        ),
                        spk[:],
                    )
    nc.compile()
    return nc


_cached_nc = None


def _get_nc():
    global _cached_nc
    if _cached_nc is None:
        _cached_nc = build_nc()
    return _cached_nc


def _shard(x):
    """Full [T, B, D] -> list of per-core [P, T, F] contiguous arrays."""
    in_maps = []
    for c in range(NCORES):
        xc = x[:, c * BL : (c + 1) * BL, :].reshape(T, P, F).transpose(1, 0, 2)
        in_maps.append({"x": np.ascontiguousarray(xc)})
    return in_maps


def _gather(results):
    """Per-core [P, T, F] uint8 outputs -> full [T, B, D] f32 (exact)."""
    outs = [
        np.asarray(results[c]["out"])
        .astype(np.float32)
        .transpose(1, 0, 2)
        .reshape(T, BL, D)
        for c in range(NCORES)
    ]
    return np.concatenate(outs, axis=1)


def run(x, trace=False, **kw):
    """Run on the 8 NeuronCores; returns (output, BassKernelResults)."""
    x = np.ascontiguousarray(np.asarray(x, dtype=np.float32))
    assert x.shape == (T, B, D), x.shape
    nc = _get_nc()
    res = run_bass_kernel_spmd(
        nc, _shard(x), core_ids=list(range(NCORES)), trace=trace, **kw
    )
    return _gather(res.results), res


def kernel(x: np.ndarray) -> np.ndarray:
    out, _ = run(x)
    return out


# revision 11
# speedup vs baseline: 6.9469x; 1.0063x over previous
"""LIF spiking-neuron recurrence on Trainium2 (8 NeuronCores).

Reference semantics (TAU=1, THRESH=1, f32):
    mem = 0
    for t in range(T):
        mem = mem + x[t]
        spike[t] = (mem >= 1.0) ? 1.0 : 0.0
        mem = mem * (1 - spike[t])        # hard reset

Sharding: data-parallel over the batch axis (B=128 -> 16 rows/core).
Per-core layout: the [T, 16, 16384] shard is viewed as [T, 128, 2048]
(partition-major within a timestep slab) and pre-transposed on the host
to [128, T, 2048] so each partition's DMA runs are contiguous.

Engine mapping per timestep (tile [128, 2048] f32):
    DVE : tmp = mem + x_t            (tensor_tensor add, 1x, ~2.29us)
    ACT : d = Sqrt(tmp + (-1))       (NaN iff tmp < 1; affine is exact)
    ACT : spike = Is_finite(d)       (exact 1.0/0.0, written as bf16)
    DVE : mem = (tmp < 1) * tmp      (scalar_tensor_tensor, 1x, ~2.29us)
The ACT spike route was probed exact on HW for all threshold edge
cases (ties, +-1ulp); GpSimd is kept idle (f32 elementwise there runs
~15-30x below DVE and its shared-port lock stalls DVE). Spikes are
stored as uint8 (0/1 exact, probed) cutting store traffic 4x; the
host upcasts. DMAs are HWDGE (loads on SP ring, stores on ACT ring),
CHUNK timesteps per transfer; the first group loads per-step (1MB) so
compute starts early, and the last group stores per-step to shorten
the tail. Step 0 uses x_0 directly (mem starts at 0); the final
step's reset is dead code and skipped.
"""

import numpy as np

try:
    import concourse  # noqa: F401
except ImportError:  # pragma: no cover
    import sys

    for _p in ("/opt/trn_rl_repo", "/root/.axon_site/_ro/trn_rl_repo"):
        if _p not in sys.path:
            sys.path.insert(0, _p)

from concourse import bacc, mybir
from concourse.bass_utils import run_bass_kernel_spmd
from concourse.mybir import ActivationFunctionType as AF
from concourse.mybir import AluOpType
from concourse.tile import TileContext

T, B, D = 64, 128, 16384
NCORES = 8
BL = B // NCORES  # 16 batch rows per core
P = 128  # SBUF partitions
F = (BL * D) // P  # 2048 free elements per timestep slab
CHUNK = 4  # timesteps per DMA transfer


def build_nc(
    t_steps=T, f_free=F, chunk=CHUNK, x_bufs=4, s_bufs=2, t_bufs=3, d_bufs=2
):
    """Build + compile the per-core Bass program (identical on all cores)."""
    assert t_steps % chunk == 0
    f32 = mybir.dt.float32
    u8 = mybir.dt.uint8
    nc = bacc.Bacc(
        "TRN2", target_bir_lowering=False, debug=False, num_devices=NCORES
    )
    x_ext = nc.dram_tensor("x", [P, t_steps, f_free], f32, kind="ExternalInput")
    out_ext = nc.dram_tensor(
        "out", [P, t_steps, f_free], u8, kind="ExternalOutput"
    )
    n_groups = t_steps // chunk
    with TileContext(nc) as tc:
        with (
            tc.tile_pool(name="xp", bufs=x_bufs) as xp,
            tc.tile_pool(name="sp", bufs=s_bufs) as sp,
            tc.tile_pool(name="tp", bufs=t_bufs) as tp,
            tc.tile_pool(name="dp", bufs=d_bufs) as dp,
            tc.tile_pool(name="mp", bufs=1) as mp,
        ):
            mem = mp.tile([P, f_free], f32)
            bm1 = mp.tile([P, 1], f32, name="bm1")
            nc.vector.memset(bm1[:], -1.0)
            for g in range(n_groups):
                xt = xp.tile([P, chunk * f_free], f32, name="xt")
                xv = x_ext[:, g * chunk : (g + 1) * chunk, :]
                # per-step loads: slice-level deps let each TT start as
                # soon as its own 1MB lands instead of the whole 4MB
                for j in range(chunk):
                    nc.sync.dma_start(
                        xt[:, j * f_free : (j + 1) * f_free], xv[:, j, :]
                    )
                spk = sp.tile([P, chunk * f_free], u8, name="spk")
                for j in range(chunk):
                    t = g * chunk + j
                    xs = xt[:, j * f_free : (j + 1) * f_free]
                    ss = spk[:, j * f_free : (j + 1) * f_free]
                    if t == 0:
                        pre = xs  # mem==0: pre-reset membrane is just x_0
                    else:
                        tmp = tp.tile([P, f_free], f32, name="tmp")
                        nc.vector.tensor_tensor(
                            tmp[:], mem[:], xs, AluOpType.add
                        )
                        pre = tmp[:]
                    # spike = Is_finite(Sqrt(pre - 1)): NaN iff pre < 1
                    d = dp.tile([P, f_free], f32, name="d")
                    nc.scalar.activation(
                        d[:], pre, AF.Sqrt, bias=bm1[:], scale=1.0
                    )
                    nc.scalar.activation(
                        ss, d[:], AF.Is_finite, bias=0.0, scale=1.0
                    )
                    if t < t_steps - 1:  # last reset is dead code
                        nc.vector.scalar_tensor_tensor(
                            mem[:], pre, 1.0, pre, AluOpType.is_lt, AluOpType.mult
                        )
                    if g == n_groups - 1:
                        # per-step stores so the tail drains quickly
                        nc.scalar.dma_start(
                            out_ext[:, g * chunk + j, :], ss
                        )
                if g < n_groups - 1:
                    nc.scalar.dma_start(
                        out_ext[:, g * chunk : (g + 1) * chunk, :].rearrange(
                            "p t f -> p (t f)"
                        ),
                        spk[:],
                    )
    nc.compile()
    return nc


_cached_nc = None


def _get_nc():
    global _cached_nc
    if _cached_nc is None:
        _cached_nc = build_nc()
    return _cached_nc


def _shard(x):
    """Full [T, B, D] -> list of per-core [P, T, F] contiguous arrays."""
    in_maps = []
    for c in range(NCORES):
        xc = x[:, c * BL : (c + 1) * BL, :].reshape(T, P, F).transpose(1, 0, 2)
        in_maps.append({"x": np.ascontiguousarray(xc)})
    return in_maps


def _gather(results):
    """Per-core [P, T, F] uint8 outputs -> full [T, B, D] f32 (exact)."""
    outs = [
        np.asarray(results[c]["out"])
        .astype(np.float32)
        .transpose(1, 0, 2)
        .reshape(T, BL, D)
        for c in range(NCORES)
    ]
    return np.concatenate(outs, axis=1)


def run(x, trace=False, **kw):
    """Run on the 8 NeuronCores; returns (output, BassKernelResults)."""
    x = np.ascontiguousarray(np.asarray(x, dtype=np.float32))
    assert x.shape == (T, B, D), x.shape
    nc = _get_nc()
    res = run_bass_kernel_spmd(
        nc, _shard(x), core_ids=list(range(NCORES)), trace=trace, **kw
    )
    return _gather(res.results), res


def kernel(x: np.ndarray) -> np.ndarray:
    out, _ = run(x)
    return out
